# revision 8
# baseline (speedup 1.0000x reference)
"""Trainium2 Bass kernel for nn_CrossAttention_15006615733765.

Mathematical structure: the reference broadcasts a per-batch context vector
(B, CTX_DIM) to every spatial position before projecting to K/V.  All keys
within a batch are therefore identical, softmax over the key axis is exactly
uniform, and the attention output equals V itself (mean of identical rows).
The whole module collapses to

    out[b, c, h, w] = ((context[b] @ Wv) @ Wo + bo)[c]

independent of x, Wq and Wk.  The kernel computes the two small matmuls on
the tensor engine and materializes the broadcast output shard per core.

Sharding: the 512 output channels are split across 8 cores (64 each).  Each
core loads the full Wv (needed for the 512-wide contraction), its Wo column
shard, computes o = (ctx @ Wv) @ Wo_shard + bo_shard -> [4, 64], replicates
that 1 KiB row across all 128 SBUF partitions via a ones-matmul on the PE,
and DMAs the replicated row to its [2304, 4, 64] output shard with a
stride-0 source view.
"""

import numpy as np

import concourse.bacc as bacc
import concourse.mybir as mybir
import concourse.tile as tile
from concourse.bass_utils import run_bass_kernel_spmd

B, DIM, CTX_DIM = 4, 512, 768
H = W = 48
NPOS = H * W            # 2304 spatial positions
NCORES = 8
CPC = DIM // NCORES     # 64 output channels per core
P = 128                 # SBUF partitions
KC = CTX_DIM // P       # 6 k-chunks for the ctx contraction
KD = DIM // P           # 4 k-chunks for the dim contraction
NREP = NPOS // P        # 18 row-replicas per partition
ROW = B * CPC           # 256 floats: one spatial position's (b, c) row
F32 = mybir.dt.float32

_CACHE: dict = {}


def _build_nc():
    nc = bacc.Bacc("TRN2", target_bir_lowering=False, debug=False, num_devices=NCORES)

    # Host-pre-chunked layouts: leading dim = SBUF partition.
    ctxc = nc.dram_tensor("ctxc", [P, KC, B], F32, kind="ExternalInput")
    wvc = nc.dram_tensor("wvc", [P, KC, DIM], F32, kind="ExternalInput")
    woc = nc.dram_tensor("woc", [P, KD, CPC], F32, kind="ExternalInput")
    boc = nc.dram_tensor("boc", [1, ROW], F32, kind="ExternalInput")
    outd = nc.dram_tensor("outd", [NPOS, ROW], F32, kind="ExternalOutput")

    with tile.TileContext(nc) as tc:
        with (
            tc.tile_pool(name="sb", bufs=1) as sb,
            tc.tile_pool(name="ps", bufs=1, space="PSUM") as ps,
        ):
            ctx_sb = sb.tile([P, KC, B], F32)
            wo_sb = sb.tile([P, KD, CPC], F32)
            # rowt row 0: gathered o row; row 1: bias row (from host).
            rowt = sb.tile([2, ROW], F32)
            ones2 = sb.tile([2, P], F32)
            wv_sb = sb.tile([P, KC, DIM], F32)

            nc.sync.dma_start(out=ctx_sb[:], in_=ctxc[:])
            nc.sync.dma_start(out=wo_sb[:], in_=woc[:])
            nc.sync.dma_start(out=rowt[1:2, :], in_=boc[:])
            nc.any.memset(ones2[:], 1.0)
            # Per-k-chunk loads so stage-1 matmuls can start on chunk 0
            # while later chunks are still in flight.
            for k in range(KC):
                nc.sync.dma_start(out=wv_sb[:, k, :], in_=wvc[:, k, :])

            # Stage 1: tT[d, b] = sum_c Wv[c, d] * ctx[b, c], d on partitions.
            pt = [
                ps.tile([P, B], F32, name=f"pt{m}", tag=f"pt{m}") for m in range(KD)
            ]
            tT_sb = sb.tile([P, KD, B], F32)
            for k in range(KC):
                for m in range(KD):
                    nc.tensor.matmul(
                        pt[m][:],
                        wv_sb[:, k, m * P:(m + 1) * P],
                        ctx_sb[:, k, :],
                        start=(k == 0),
                        stop=(k == KC - 1),
                    )
            for m in range(KD):
                nc.scalar.copy(tT_sb[:, m, :], pt[m][:])

            # Stage 2: o[b, c] = sum_d t[b, d] * Wo[d, c] -> PSUM [B, CPC].
            po = ps.tile([B, CPC], F32, tag="po")
            for k in range(KD):
                nc.tensor.matmul(
                    po[:],
                    tT_sb[:, k, :],
                    wo_sb[:, k, :],
                    start=(k == 0),
                    stop=(k == KD - 1),
                )
            # Gather the [B, CPC] result onto partition 0 as one flat row.
            o_sb = sb.tile([B, CPC], F32)
            nc.scalar.copy(o_sb[:], po[:])
            for b in range(B):
                nc.sync.dma_start(
                    out=rowt[0:1, b * CPC:(b + 1) * CPC], in_=o_sb[b:b + 1, :]
                )

            # Stage 3: K=2 ones-matmul broadcasts (o_row + bias_row) to all
            # 128 partitions in one shot.
            prep = ps.tile([P, ROW], F32, tag="prep")
            nc.tensor.matmul(
                prep[:], ones2[:], rowt[:], start=True, stop=True
            )
            rep_sb = sb.tile([P, ROW], F32)
            nc.vector.tensor_copy(rep_sb[:], prep[:])

            # Output: outd[r*128 + p, n] = rep_sb[p, n] for r in 0..18.
            out_view = outd.rearrange("(r p) n -> p r n", p=P)
            src_view = rep_sb[:, None, :].broadcast_to((P, NREP, ROW))
            nc.sync.dma_start(out=out_view, in_=src_view)

    nc.compile()
    return nc


def _get_nc():
    if "nc" not in _CACHE:
        _CACHE["nc"] = _build_nc()
    return _CACHE["nc"]


def _prepare_in_maps(context, Wv, Wo, bo):
    context = np.ascontiguousarray(context, dtype=np.float32)
    Wv = np.ascontiguousarray(Wv, dtype=np.float32)
    Wo = np.ascontiguousarray(Wo, dtype=np.float32)
    bo = np.ascontiguousarray(bo, dtype=np.float32)

    ctxc = np.ascontiguousarray(context.T.reshape(KC, P, B).transpose(1, 0, 2))
    wvc = np.ascontiguousarray(Wv.reshape(KC, P, DIM).transpose(1, 0, 2))
    wo_chunk = Wo.reshape(KD, P, DIM).transpose(1, 0, 2)  # [P, KD, DIM]
    in_maps = []
    for i in range(NCORES):
        woc = np.ascontiguousarray(wo_chunk[:, :, i * CPC:(i + 1) * CPC])
        boc = np.ascontiguousarray(np.tile(bo[i * CPC:(i + 1) * CPC], B))
        in_maps.append(
            {"ctxc": ctxc, "wvc": wvc, "woc": woc, "boc": boc.reshape(1, ROW)}
        )
    return in_maps


def _unshard(results):
    # Unshard: full[b, i*64 + c, p] = shard_i[p, b*64 + c].
    shards = np.stack([r["outd"] for r in results], axis=0)  # [8, NPOS, ROW]
    shards = shards.reshape(NCORES, NPOS, B, CPC)
    out = shards.transpose(2, 0, 3, 1).reshape(B, DIM, H, W)
    return np.ascontiguousarray(out)


def kernel(x, context, Wq, Wk, Wv, Wo, bo):
    del x, Wq, Wk  # output is mathematically independent of these
    nc = _get_nc()
    in_maps = _prepare_in_maps(context, Wv, Wo, bo)
    results = run_bass_kernel_spmd(nc, in_maps, list(range(NCORES))).results
    return _unshard(results)


# revision 9
# speedup vs baseline: 1.2413x; 1.2413x over previous
"""Trainium2 Bass kernel for nn_CrossAttention_15006615733765.

Mathematical structure: the reference broadcasts a per-batch context vector
(B, CTX_DIM) to every spatial position before projecting to K/V.  All keys
within a batch are therefore identical, softmax over the key axis is exactly
uniform, and the attention output equals V itself (mean of identical rows).
The whole module collapses to

    out[b, c, h, w] = ((context[b] @ Wv) @ Wo + bo)[c]

independent of x, Wq and Wk.  The kernel computes the two small matmuls on
the tensor engine and materializes the broadcast output shard per core.

Sharding: the 512 output channels are split across 8 cores (64 each).

Per-core pipeline (all fp32, exact):
  1. t[b, d]  = sum_c ctx[b, c] Wv[c, d]   -- 6 fat matmuls, ctx stationary
  2. tT       = PE-transpose of t (4 chunks of [4,128] -> [128,4])
  3. o[b, c]  = sum_d t[b, d] Wo[d, c]     -- 4 matmuls, tT stationary
  4. prep[p, (b,c)] = o[b, c] + bo[c]      -- 4 selector matmuls replicate
     the (b,c) row to all 128 partitions and fold in the bias
  5. one stride-0 DMA writes the [2304, 256] output shard
"""

import numpy as np

import concourse.bacc as bacc
import concourse.mybir as mybir
import concourse.tile as tile
from concourse.bass_utils import run_bass_kernel_spmd

B, DIM, CTX_DIM = 4, 512, 768
H = W = 48
NPOS = H * W            # 2304 spatial positions
NCORES = 8
CPC = DIM // NCORES     # 64 output channels per core
P = 128                 # SBUF partitions
KC = CTX_DIM // P       # 6 k-chunks for the ctx contraction
KD = DIM // P           # 4 k-chunks for the dim contraction
NREP = NPOS // P        # 18 row-replicas per partition
ROW = B * CPC           # 256 floats: one spatial position's (b, c) row
F32 = mybir.dt.float32

_CACHE: dict = {}


def _build_nc():
    nc = bacc.Bacc("TRN2", target_bir_lowering=False, debug=False, num_devices=NCORES)

    # Host-pre-chunked layouts: leading dim = SBUF partition.
    ctxc = nc.dram_tensor("ctxc", [P, KC, B], F32, kind="ExternalInput")
    wvc = nc.dram_tensor("wvc", [P, KC, DIM], F32, kind="ExternalInput")
    woc = nc.dram_tensor("woc", [P, KD, CPC], F32, kind="ExternalInput")
    # selc[k, b, p] = (k == b) + (k == B): selector weights for the
    # replicate-and-bias matmuls.
    selc = nc.dram_tensor("selc", [B + 1, B, P], F32, kind="ExternalInput")
    idc = nc.dram_tensor("idc", [B, B], F32, kind="ExternalInput")
    boc = nc.dram_tensor("boc", [1, CPC], F32, kind="ExternalInput")
    outd = nc.dram_tensor("outd", [NPOS, ROW], F32, kind="ExternalOutput")

    with tile.TileContext(nc) as tc:
        with (
            tc.tile_pool(name="sb", bufs=1) as sb,
            tc.tile_pool(name="ps", bufs=1, space="PSUM") as ps,
        ):
            ctx_sb = sb.tile([P, KC, B], F32)
            wv_sb = sb.tile([P, KC, DIM], F32)
            wo_sb = sb.tile([P, KD, CPC], F32)
            sel_sb = sb.tile([B + 1, B, P], F32)
            id_sb = sb.tile([B, B], F32)
            # o5 rows 0..3: o[b, :]; row 4: bias row.
            o5_sb = sb.tile([B + 1, CPC], F32)

            # Critical-path loads first (ctx + Wv feed stage 1), small
            # later-stage constants on the scalar queue.
            nc.sync.dma_start(out=ctx_sb[:], in_=ctxc[:])
            for k in range(KC):
                nc.sync.dma_start(out=wv_sb[:, k, :], in_=wvc[:, k, :])
            nc.scalar.dma_start(out=wo_sb[:], in_=woc[:])
            nc.scalar.dma_start(out=sel_sb[:], in_=selc[:])
            nc.scalar.dma_start(out=id_sb[:], in_=idc[:])
            nc.scalar.dma_start(out=o5_sb[B:B + 1, :], in_=boc[:])

            # Stage 1: t[b, d] = sum_c ctx[b, c] Wv[c, d].  ctx chunk is the
            # stationary operand (4 columns -> fast LDWEIGHTS), Wv chunk
            # streams N=512.
            pt = ps.tile([B, DIM], F32, tag="pt")
            for k in range(KC):
                nc.tensor.matmul(
                    pt[:],
                    ctx_sb[:, k, :],
                    wv_sb[:, k, :],
                    start=(k == 0),
                    stop=(k == KC - 1),
                )
            t_sb = sb.tile([B, DIM], F32)
            nc.vector.tensor_copy(t_sb[:], pt[:])

            # Stage 2a: transpose t -> tT ([4, 512] -> 4x [128, 4]).
            ptT = ps.tile([P, KD, B], F32, tag="ptT")
            for m in range(KD):
                nc.tensor.transpose(
                    ptT[:, m, :], t_sb[:, m * P:(m + 1) * P], id_sb[:]
                )
            tT_sb = sb.tile([P, KD, B], F32)
            nc.vector.tensor_copy(tT_sb[:], ptT[:])

            # Stage 2b: o[b, c] = sum_d t[b, d] Wo[d, c].
            po = ps.tile([B, CPC], F32, tag="po")
            for m in range(KD):
                nc.tensor.matmul(
                    po[:],
                    tT_sb[:, m, :],
                    wo_sb[:, m, :],
                    start=(m == 0),
                    stop=(m == KD - 1),
                )
            nc.vector.tensor_copy(o5_sb[:B, :], po[:])

            # Stage 3: selector matmuls replicate row (b, c) to all 128
            # partitions and add the bias row in the same contraction.
            prep = ps.tile([P, B, CPC], F32, tag="prep")
            for b in range(B):
                nc.tensor.matmul(
                    prep[:, b, :],
                    sel_sb[:, b, :],
                    o5_sb[:, :],
                    start=True,
                    stop=True,
                )
            rep_sb = sb.tile([P, ROW], F32)
            nc.vector.tensor_copy(rep_sb[:], prep[:].rearrange("p b c -> p (b c)"))

            # Output: outd[r*128 + p, n] = rep_sb[p, n] for r in 0..18.
            out_view = outd.rearrange("(r p) n -> p r n", p=P)
            src_view = rep_sb[:, None, :].broadcast_to((P, NREP, ROW))
            nc.sync.dma_start(out=out_view, in_=src_view)

    nc.compile()
    return nc


def _get_nc():
    if "nc" not in _CACHE:
        _CACHE["nc"] = _build_nc()
    return _CACHE["nc"]


def _prepare_in_maps(context, Wv, Wo, bo):
    context = np.ascontiguousarray(context, dtype=np.float32)
    Wv = np.ascontiguousarray(Wv, dtype=np.float32)
    Wo = np.ascontiguousarray(Wo, dtype=np.float32)
    bo = np.ascontiguousarray(bo, dtype=np.float32)

    ctxc = np.ascontiguousarray(context.T.reshape(KC, P, B).transpose(1, 0, 2))
    wvc = np.ascontiguousarray(Wv.reshape(KC, P, DIM).transpose(1, 0, 2))
    wo_chunk = Wo.reshape(KD, P, DIM).transpose(1, 0, 2)  # [P, KD, DIM]

    selc = np.zeros((B + 1, B, P), dtype=np.float32)
    for b in range(B):
        selc[b, b, :] = 1.0
        selc[B, b, :] = 1.0
    idc = np.eye(B, dtype=np.float32)

    in_maps = []
    for i in range(NCORES):
        woc = np.ascontiguousarray(wo_chunk[:, :, i * CPC:(i + 1) * CPC])
        boc = np.ascontiguousarray(bo[i * CPC:(i + 1) * CPC]).reshape(1, CPC)
        in_maps.append(
            {
                "ctxc": ctxc,
                "wvc": wvc,
                "woc": woc,
                "selc": selc,
                "idc": idc,
                "boc": boc,
            }
        )
    return in_maps


def _unshard(results):
    # Unshard: full[b, i*64 + c, p] = shard_i[p, b*64 + c].
    shards = np.stack([r["outd"] for r in results], axis=0)  # [8, NPOS, ROW]
    shards = shards.reshape(NCORES, NPOS, B, CPC)
    out = shards.transpose(2, 0, 3, 1).reshape(B, DIM, H, W)
    return np.ascontiguousarray(out)


def kernel(x, context, Wq, Wk, Wv, Wo, bo):
    del x, Wq, Wk  # output is mathematically independent of these
    nc = _get_nc()
    in_maps = _prepare_in_maps(context, Wv, Wo, bo)
    results = run_bass_kernel_spmd(nc, in_maps, list(range(NCORES))).results
    return _unshard(results)
